# revision 2
# baseline (speedup 1.0000x reference)
"""Trainium2 Bass kernel for nn_Attention_80367428043370.

Math (the reference reduces to this):
  q  = W @ x  (1x1 conv per spatial position)
  kv = conv1x1(pad(x)) = pad(q)
  logits[c,h,w,di,dj] = q[c,h,w] * (kvp[c,h+di,w+dj] + rel[c,di,dj])
  attn = softmax over dj (size 3); out = sum_di attn[...,di,di]*kvp[c,h+di,w+di]

Per-row softmax-ratio form (6 exps instead of 9):
  sigma_di = 1/(1 + sum_{dj!=di} exp(q*(kvp[.,h+di,w+dj]-kvp[.,h+di,w+di]+dr)))
  out      = sum_di sigma_di * kvp[.,h+di,w+di],  dr = rel[c,di,dj]-rel[c,di,di]

Column differences of kvp are shift-shared planes G1/G2.  All elementwise
work is bf16 (DVE 2x modes) except the reciprocal (internal f32).  A custom
DVE op fuses s=1+e0+e1 with an approximate reciprocal (seed + 1 NR step).
Even/odd column-phase copies of the padded plane keep every strided read
4B-aligned so bf16 2x perf modes engage.

Sharding: data-parallel over batch B=8 -> one batch item per NeuronCore.
"""

import sys

for p in ("/opt/trn_rl_repo", "/opt/pypackages"):
    if p not in sys.path:
        sys.path.insert(0, p)

import numpy as np

import concourse.bass as bass

import concourse.bacc as bacc
import concourse.mybir as mybir
import concourse.tile as tile
import concourse.dve_ops as dve_ops_mod
from concourse.dve_ops import DveOp
from concourse.dve_spec import Spec, Src0, Src1, C0, C1, C2, Bin, AluOp
from concourse.bass_utils import run_bass_kernel_spmd

F32 = mybir.dt.float32
BF16 = mybir.dt.bfloat16
AF = mybir.ActivationFunctionType
OP = mybir.AluOpType

B, C, H, W = 8, 256, 64, 64
HW = H * W
NCORES = 8

RECIP_C0 = -0.23549792
RECIP_C1 = 2.0017324


def _register_sumrecip():
    """sigma = approx 1/(imm2 + in0 + in1): BITWISE_NOT seed + 1 NR step."""
    name = "SUM1_RECIP_NR1"
    if name in dve_ops_mod._SUB_OPCODE_FOR_NAME:
        return next(o for o in dve_ops_mod.OPS if o.name == name)
    _s = (Src0 + Src1) + C2
    _not = Bin(AluOp.BITWISE_NOT, _s, _s)
    _y0 = _not * C0
    body = _y0 * (C1 - _s * _y0)

    def _ref(in0, in1, s0, s1, imm2):
        s = in0.astype(np.float32) + in1.astype(np.float32) + np.float32(imm2)
        nx = (~s.view(np.int32)).view(np.float32)
        y0 = nx * np.float32(s0)
        return (y0 * (np.float32(s1) - s * y0)).astype(np.float32)

    op = DveOp(
        name,
        Spec(body=body, reference=_ref),
        subdim=False,
        uops_sha={"v3": "994dd8b3ea1c7e4c", "v4": "a9846cbf3526a936"},
    )
    dve_ops_mod.OPS.append(op)
    dve_ops_mod._SUB_OPCODE_FOR_NAME[op.name] = (
        max(dve_ops_mod._SUB_OPCODE_FOR_NAME.values()) + 1
    )
    dve_ops_mod.CUSTOM_DVE_SPECS[op.name] = op.spec
    return op


SUMRECIP = _register_sumrecip()

# pair table: (pair_idx, di, plane, row_off, col_off_in_phase, positive_sign)
# planes are phase-separated so every read starts 4B-aligned (bf16).
# arg(di,dj) = q * (sgn*G[...] + dr_pair);  neg pairs compute (G - dr)*q and
# fold the sign into the exp's scale=-1.
PAIRS = [
    (0, 0, "g1e", 0, 0, True),   # (0,1):  +G1[h+0, w+0]
    (1, 0, "g2e", 0, 0, True),   # (0,2):  +G2[h+0, w+0]
    (2, 1, "g1e", 1, 0, False),  # (1,0):  -G1[h+1, w+0]
    (3, 1, "g1o", 1, 0, True),   # (1,2):  +G1[h+1, w+1]
    (4, 2, "g2e", 2, 0, False),  # (2,0):  -G2[h+2, w+0]
    (5, 2, "g1o", 2, 0, False),  # (2,1):  -G1[h+2, w+1]
]

CH = 32  # pointwise chunk height (rows)
KVPO_MODE = "pool"  # odd-phase production: "dma" | "act" | "pool"
ACC_ENG = "pool"   # acc = m02[0]+m02[1]
ACCF_ENG = "pool"  # accf = acc+m1
M1_ENG = "pool"    # m1 = sig1*v1
M02_POOL_HC = (0,)  # hc indices whose m02 mul runs on gpsimd
WORK_BUFS = 2
MM_BUFS = 2
E2_BUFS = 0      # >0 -> e2 (ACT->DVE handoff) gets its own pool this deep
SIG_BUFS = 0     # >0 -> sig_all gets its own pool this deep
TS_ACT = (0, 3)  # pair indices whose t2 build runs on ACT (Identity affine)


def _build(reps=1):
    nc = bacc.Bacc("TRN2", target_bir_lowering=False, debug=False)

    x_ext = nc.dram_tensor("x", [C, HW], BF16, kind="ExternalInput")
    wt_ext = nc.dram_tensor("wt", [C, C], BF16, kind="ExternalInput")  # W.T [cin,cout]
    dr_ext = nc.dram_tensor("dr", [C, 6], F32, kind="ExternalInput")
    out_ext = nc.dram_tensor("out", [C, HW], BF16, kind="ExternalOutput")

    with tile.TileContext(nc) as tc:
        with (
            tc.tile_pool(name="const", bufs=1) as const,
            tc.tile_pool(name="planes", bufs=2) as planes,
            tc.tile_pool(name="psum", bufs=8, space="PSUM") as psum,
            tc.tile_pool(name="work", bufs=WORK_BUFS) as work,
            tc.tile_pool(name="mm", bufs=MM_BUFS) as mmp,
            tc.tile_pool(name="e2p", bufs=max(E2_BUFS, 1)) as e2pool,
            tc.tile_pool(name="sigp", bufs=max(SIG_BUFS, 1)) as sigpool,
        ):
            x_sb = []
            for ib in range(2):
                t = const.tile([128, HW], BF16, tag=f"x{ib}")
                # split the 2MB load into 4 column chunks so matmuls can
                # start early and queues parallelize
                engs = [nc.sync, nc.scalar, nc.gpsimd, nc.sync]
                for xc in range(4):
                    sl = slice(xc * (HW // 4), (xc + 1) * (HW // 4))
                    engs[xc].dma_start(
                        out=t[:, sl],
                        in_=x_ext.ap()[ib * 128:(ib + 1) * 128, sl])
                x_sb.append(t)
            wt_sb = []
            for ib in range(2):
                t = const.tile([128, C], BF16, tag=f"wt{ib}")
                nc.sync.dma_start(out=t, in_=wt_ext.ap()[ib * 128:(ib + 1) * 128, :])
                wt_sb.append(t)
            dr_sb = []
            for ot in range(2):
                t = const.tile([128, 6], F32, tag=f"dr{ot}")
                nc.sync.dma_start(out=t, in_=dr_ext.ap()[ot * 128:(ot + 1) * 128, :])
                dr_sb.append(t)

            import contextlib
            _loop = tc.For_i(0, reps, 1) if reps > 1 else contextlib.nullcontext()
            with _loop:
                _body(nc, tc, const, planes, psum, work, mmp,
                      e2pool if E2_BUFS else work,
                      sigpool if SIG_BUFS else work,
                      x_sb, wt_sb, dr_sb, x_ext, wt_ext, dr_ext, out_ext)

    nc.compile()
    return nc


def _body(nc, tc, const, planes, psum, work, mmp, e2pool, sigpool,
          x_sb, wt_sb, dr_sb, x_ext, wt_ext, dr_ext, out_ext):
            kvpe_ot, kvpo_ot = [], []
            for ot in range(2):
                # padded q plane, even phase: kvpe[:, r, j] = kvp[r, j]
                kvpe = planes.tile([128, H + 2, W + 2], BF16, tag="kvpe")
                # zero only the border (interior is fully overwritten)
                nc.gpsimd.memset(kvpe[:, 0, :], 0.0)
                nc.gpsimd.memset(kvpe[:, 65, :], 0.0)
                nc.gpsimd.memset(kvpe[:, 1:65, 0:1], 0.0)
                nc.gpsimd.memset(kvpe[:, 1:65, 65:66], 0.0)
                # odd phase: kvpo[:, r, j] = kvp[r, j+1]
                kvpo = planes.tile([128, H + 2, W + 2], BF16, tag="kvpo")

                # kvpo border: only row 0 / row 65 / col 64 matter (cols
                # [0,65) are written by the odd-phase copies below except the
                # border rows; col 64 = kvp col 65 = 0)
                nc.gpsimd.memset(kvpo[:, 0, :], 0.0)
                nc.gpsimd.memset(kvpo[:, 65, :], 0.0)
                nc.gpsimd.memset(kvpo[:, 1:65, 64:66], 0.0)

                # q = wt.T @ x in PSUM chunks of 512.  ib is the OUTER loop so
                # the stationary weights stay loaded for 8 consecutive
                # matmuls; 8 psum banks accumulate in parallel.
                pss = []
                for ck in range(HW // 512):
                    ps = psum.tile([128, 512], F32, tag="ps", name=f"ps{ot}_{ck}")
                    pss.append(ps)
                # ot0: ck-outer so each psum chunk finishes after 2
                # matmuls and the first copies/pointwise start early; ot1:
                # ib-outer (stationary reuse) since it overlaps ot0 pointwise
                mm_order = ([(ib, ck) for ck in range(HW // 512)
                             for ib in range(2)] if ot == 0 else
                            [(ib, ck) for ib in range(2)
                             for ck in range(HW // 512)])
                for (ib, ck) in mm_order:
                    nc.tensor.matmul(
                        pss[ck],
                        wt_sb[ib][:, ot * 128:(ot + 1) * 128],
                        x_sb[ib][:, ck * 512:(ck + 1) * 512],
                        start=(ib == 0),
                        stop=(ib == 1),
                    )
                for ck in range(HW // 512):
                    # even-phase copy into kvpe interior band
                    nc.scalar.activation(
                        out=kvpe[:, 1 + ck * 8:1 + (ck + 1) * 8, 1:65],
                        in_=pss[ck].rearrange("p (r c) -> p r c", r=8),
                        func=AF.Copy,
                    )
                    # odd phase band
                    if KVPO_MODE == "dma":
                        nc.sync.dma_start(
                            out=kvpo[:, 1 + ck * 8:1 + (ck + 1) * 8, 0:65],
                            in_=kvpe[:, 1 + ck * 8:1 + (ck + 1) * 8, 1:66],
                        )
                    elif KVPO_MODE == "act":
                        nc.scalar.activation(
                            out=kvpo[:, 1 + ck * 8:1 + (ck + 1) * 8, 0:64],
                            in_=pss[ck].rearrange("p (r c) -> p r c", r=8),
                            func=AF.Copy,
                        )
                    else:  # pool
                        nc.gpsimd.tensor_copy(
                            out=kvpo[:, 1 + ck * 8:1 + (ck + 1) * 8, 0:65],
                            in_=kvpe[:, 1 + ck * 8:1 + (ck + 1) * 8, 1:66],
                        )
                kvpe_ot.append(kvpe)
                kvpo_ot.append(kvpo)

                for hc in range(H // CH):
                    h0 = hc * CH
                    # difference planes for this chunk's row band
                    # [h0, h0+CH+2) (bf16 2x: all reads 4B-aligned)
                    # g1e[r,j] = kvp[r,j+1]-kvp[r,j]
                    # g1o[r,j] = kvp[r,j+2]-kvp[r,j+1]
                    # g2e[r,j] = kvp[r,j+2]-kvp[r,j]
                    kvpe_b = kvpe[:, h0:h0 + CH + 2, :]
                    kvpo_b = kvpo[:, h0:h0 + CH + 2, :]
                    g1e = work.tile([128, CH + 2, W + 2], BF16, tag="g1e")
                    g1o = work.tile([128, CH + 2, W + 2], BF16, tag="g1o")
                    g2e = work.tile([128, CH + 2, W + 2], BF16, tag="g2e")
                    nc.vector.tensor_sub(g1e[:, :, 0:64], kvpo_b[:, :, 0:64],
                                         kvpe_b[:, :, 0:64])
                    nc.vector.tensor_sub(g1o[:, :, 0:64], kvpe_b[:, :, 2:66],
                                         kvpo_b[:, :, 0:64])
                    nc.vector.tensor_sub(g2e[:, :, 0:64], kvpe_b[:, :, 2:66],
                                         kvpe_b[:, :, 0:64])
                    gmap = {"g1e": g1e, "g1o": g1o, "g2e": g2e}
                    # q[h,w] = kvp[h+1, w+1] = kvpo[h+1, w]
                    qv = kvpo[:, 1 + h0:1 + h0 + CH, 0:64]
                    sig_all = sigpool.tile([128, 3, CH, W], BF16, tag="sig")
                    for di in range(3):
                        # both pairs of this row batched through one tile:
                        # t2[:,k] = +-(G_k) + dr_k, then one broadcast mul by
                        # q, one exp, one fused sum+reciprocal.
                        row_pairs = [pr for pr in PAIRS if pr[1] == di]
                        t2 = work.tile([128, 2, CH, W], BF16, tag="t2")
                        for k, (p, pdi, pl, ro, co, pos) in enumerate(row_pairs):
                            gview = gmap[pl][:, ro:ro + CH, co:co + W]
                            if p in TS_ACT:
                                # +-G + dr as an ACT affine: Identity takes a
                                # per-partition bias AP and a scale immediate
                                nc.scalar.activation(
                                    out=t2[:, k], in_=gview, func=AF.Identity,
                                    scale=1.0 if pos else -1.0,
                                    bias=dr_sb[ot][:, p:p + 1],
                                )
                            elif pos:
                                nc.vector.tensor_scalar_add(
                                    t2[:, k], gview, dr_sb[ot][:, p:p + 1])
                            else:
                                # (G - dr) * -1 = dr - G, one 4x ts op
                                nc.vector.tensor_scalar(
                                    out=t2[:, k], in0=gview,
                                    scalar1=dr_sb[ot][:, p:p + 1],
                                    scalar2=-1.0,
                                    op0=OP.subtract, op1=OP.mult,
                                )
                        # one batched mul; q is free-dim-broadcast (stride 0)
                        # across the two pairs (HW-verified at 2x)
                        qb = bass.AP(tensor=qv.tensor, offset=qv.offset,
                                     ap=[qv.ap[0], [0, 2], qv.ap[1], qv.ap[2]])
                        a2 = work.tile([128, 2, CH, W], BF16, tag="a2")
                        nc.vector.tensor_mul(a2, t2, qb)
                        e2 = e2pool.tile([128, 2, CH, W], BF16, tag="e2")
                        nc.scalar.activation(out=e2, in_=a2, func=AF.Exp)
                        nc.vector._custom_dve(
                            SUMRECIP,
                            out=sig_all[:, di].rearrange("p r c -> p (r c)"),
                            in0=e2[:, 0].rearrange("p r c -> p (r c)"),
                            in1=e2[:, 1].rearrange("p r c -> p (r c)"),
                            s0=RECIP_C0, s1=RECIP_C1, imm2=1.0,
                        )
                    # final: out = sum_di sig_di * kvp[h+di, w+di].
                    # di 0 and 2 both read kvpe with a uniform 2*66+2 = 134
                    # element offset between them -> one batched 2x TT mul.
                    s02 = sig_all[:, 0:3:2]
                    v0 = kvpe[:, h0:h0 + CH, 0:64]
                    v02 = bass.AP(tensor=v0.tensor, offset=v0.offset,
                                  ap=[v0.ap[0], [134, 2], v0.ap[1], v0.ap[2]])
                    eng = {"pool": nc.gpsimd, "dve": nc.vector}
                    m02 = mmp.tile([128, 2, CH, W], BF16, tag="m02")
                    m02eng = nc.gpsimd if hc in M02_POOL_HC else nc.vector
                    m02eng.tensor_mul(m02, s02, v02)
                    m1 = mmp.tile([128, CH, W], BF16, tag="m1")
                    eng[M1_ENG].tensor_mul(m1, sig_all[:, 1],
                                           kvpo[:, 1 + h0:1 + h0 + CH, 0:64])
                    acc = mmp.tile([128, CH, W], BF16, tag="acc")
                    eng[ACC_ENG].tensor_add(acc, m02[:, 0], m02[:, 1])
                    accf = mmp.tile([128, CH, W], BF16, tag="accf")
                    eng[ACCF_ENG].tensor_add(accf, acc, m1)
                    nc.sync.dma_start(
                        out=out_ext.ap()[ot * 128:(ot + 1) * 128,
                                         h0 * W:(h0 + CH) * W],
                        in_=accf.rearrange("p r c -> p (r c)"),
                    )





_CACHE = {}


def _get_nc():
    if "nc" not in _CACHE:
        _CACHE["nc"] = _build()
    return _CACHE["nc"]


def _prep_in_maps(x, W_, rel):
    import ml_dtypes
    bf16 = ml_dtypes.bfloat16
    wt = np.ascontiguousarray(W_.T.astype(bf16))  # [cin, cout]
    r = rel.reshape(C, 3, 3).astype(np.float32)
    pairs = [(0, 1), (0, 2), (1, 0), (1, 2), (2, 0), (2, 1)]
    dr = np.stack([r[:, di, dj] - r[:, di, di] for (di, dj) in pairs], axis=1)
    dr = np.ascontiguousarray(dr.astype(np.float32))  # [C, 6]
    in_maps = []
    for c in range(NCORES):
        in_maps.append({
            "x": np.ascontiguousarray(x[c].reshape(C, HW).astype(bf16)),
            "wt": wt,
            "dr": dr,
        })
    return in_maps


def kernel(x, W, rel):
    nc = _get_nc()
    in_maps = _prep_in_maps(x, W, rel)
    res = run_bass_kernel_spmd(nc, in_maps, core_ids=list(range(NCORES)))
    out = np.stack([
        res.results[c]["out"].astype(np.float32).reshape(C, H, 64)
        for c in range(NCORES)
    ])
    return out.astype(np.float32)



# revision 3
# speedup vs baseline: 1.9179x; 1.9179x over previous
"""Trainium2 Bass kernel for nn_Attention_80367428043370.

Math (the reference reduces to this):
  q  = W @ x  (1x1 conv per spatial position)
  kv = conv1x1(pad(x)) = pad(q)
  logits[c,h,w,di,dj] = q[c,h,w] * (kvp[c,h+di,w+dj] + rel[c,di,dj])
  attn = softmax over dj (size 3); out = sum_di attn[...,di,di]*kvp[c,h+di,w+di]

Per-row softmax-ratio form (6 exps instead of 9):
  sigma_di = 1/(1 + sum_{dj!=di} exp(q*(kvp[.,h+di,w+dj]-kvp[.,h+di,w+di]+dr)))
  out      = sum_di sigma_di * kvp[.,h+di,w+di],  dr = rel[c,di,dj]-rel[c,di,di]

Column differences of kvp are shift-shared planes G1/G2.  All elementwise
work is bf16 (DVE 2x modes) except the reciprocal (internal f32).  A custom
DVE op fuses s=1+e0+e1 with an approximate reciprocal (seed + 1 NR step).
Even/odd column-phase copies of the padded plane keep every strided read
4B-aligned so bf16 2x perf modes engage.

Sharding: data-parallel over batch B=8 -> one batch item per NeuronCore.
"""

import sys

for p in ("/opt/trn_rl_repo", "/opt/pypackages"):
    if p not in sys.path:
        sys.path.insert(0, p)

import numpy as np

import concourse.bass as bass

import concourse.bacc as bacc
import concourse.mybir as mybir
import concourse.tile as tile
import concourse.dve_ops as dve_ops_mod
from concourse.dve_ops import DveOp
from concourse.dve_spec import Spec, Src0, Src1, C0, C1, C2, Bin, AluOp
from concourse.bass_utils import run_bass_kernel_spmd

F32 = mybir.dt.float32
BF16 = mybir.dt.bfloat16
AF = mybir.ActivationFunctionType
OP = mybir.AluOpType

B, C, H, W = 8, 256, 64, 64
HW = H * W
NCORES = 8

RECIP_C0 = -0.23549792
RECIP_C1 = 2.0017324


def _register_sumrecip():
    """sigma = approx 1/(imm2 + in0 + in1): BITWISE_NOT seed + 1 NR step."""
    name = "SUM1_RECIP_NR1"
    if name in dve_ops_mod._SUB_OPCODE_FOR_NAME:
        return next(o for o in dve_ops_mod.OPS if o.name == name)
    _s = (Src0 + Src1) + C2
    _not = Bin(AluOp.BITWISE_NOT, _s, _s)
    _y0 = _not * C0
    body = _y0 * (C1 - _s * _y0)

    def _ref(in0, in1, s0, s1, imm2):
        s = in0.astype(np.float32) + in1.astype(np.float32) + np.float32(imm2)
        nx = (~s.view(np.int32)).view(np.float32)
        y0 = nx * np.float32(s0)
        return (y0 * (np.float32(s1) - s * y0)).astype(np.float32)

    op = DveOp(
        name,
        Spec(body=body, reference=_ref),
        subdim=False,
        uops_sha={"v3": "994dd8b3ea1c7e4c", "v4": "a9846cbf3526a936"},
    )
    dve_ops_mod.OPS.append(op)
    dve_ops_mod._SUB_OPCODE_FOR_NAME[op.name] = (
        max(dve_ops_mod._SUB_OPCODE_FOR_NAME.values()) + 1
    )
    dve_ops_mod.CUSTOM_DVE_SPECS[op.name] = op.spec
    return op


SUMRECIP = _register_sumrecip()

# pair table: (pair_idx, di, plane, row_off, col_off_in_phase, positive_sign)
# planes are phase-separated so every read starts 4B-aligned (bf16).
# arg(di,dj) = q * (sgn*G[...] + dr_pair);  neg pairs compute (G - dr)*q and
# fold the sign into the exp's scale=-1.
PAIRS = [
    (0, 0, "g1e", 0, 0, True),   # (0,1):  +G1[h+0, w+0]
    (1, 0, "g2e", 0, 0, True),   # (0,2):  +G2[h+0, w+0]
    (2, 1, "g1e", 1, 0, False),  # (1,0):  -G1[h+1, w+0]
    (3, 1, "g1o", 1, 0, True),   # (1,2):  +G1[h+1, w+1]
    (4, 2, "g2e", 2, 0, False),  # (2,0):  -G2[h+2, w+0]
    (5, 2, "g1o", 2, 0, False),  # (2,1):  -G1[h+2, w+1]
]

CH = 32  # pointwise chunk height (rows)
KVPO_MODE = "pool"  # odd-phase production: "dma" | "act" | "pool"
KVPE_ENG = "act"  # PSUM->SBUF kvpe copy (pool cannot read PSUM)
ACC_ENG = "dve"   # acc = m02[0]+m02[1]
ACCF_ENG = "dve"  # accf = acc+m1
M1_ENG = "dve"    # m1 = sig1*v1
M02_POOL_HC = ()  # hc indices whose m02 mul runs on gpsimd
WORK_BUFS = 2
MM_BUFS = 2
E2_BUFS = 0      # >0 -> e2 (ACT->DVE handoff) gets its own pool this deep
SIG_BUFS = 0     # >0 -> sig_all gets its own pool this deep
TS_ACT = (0, 3)  # pair indices whose t2 build runs on ACT (Identity affine)


def _build(reps=1):
    nc = bacc.Bacc("TRN2", target_bir_lowering=False, debug=False)

    x_ext = nc.dram_tensor("x", [C, HW], BF16, kind="ExternalInput")
    wt_ext = nc.dram_tensor("wt", [C, C], BF16, kind="ExternalInput")  # W.T [cin,cout]
    dr_ext = nc.dram_tensor("dr", [C, 6], F32, kind="ExternalInput")
    out_ext = nc.dram_tensor("out", [C, HW], BF16, kind="ExternalOutput")

    with tile.TileContext(nc) as tc:
        with (
            tc.tile_pool(name="const", bufs=1) as const,
            tc.tile_pool(name="planes", bufs=2) as planes,
            tc.tile_pool(name="psum", bufs=8, space="PSUM") as psum,
            tc.tile_pool(name="work", bufs=WORK_BUFS) as work,
            tc.tile_pool(name="mm", bufs=MM_BUFS) as mmp,
            tc.tile_pool(name="e2p", bufs=max(E2_BUFS, 1)) as e2pool,
            tc.tile_pool(name="sigp", bufs=max(SIG_BUFS, 1)) as sigpool,
        ):
            x_sb = []
            for ib in range(2):
                t = const.tile([128, HW], BF16, tag=f"x{ib}")
                # split the 2MB load into 4 column chunks so matmuls can
                # start early and queues parallelize
                engs = [nc.sync, nc.scalar, nc.gpsimd, nc.sync]
                for xc in range(4):
                    sl = slice(xc * (HW // 4), (xc + 1) * (HW // 4))
                    engs[xc].dma_start(
                        out=t[:, sl],
                        in_=x_ext.ap()[ib * 128:(ib + 1) * 128, sl])
                x_sb.append(t)
            wt_sb = []
            for ib in range(2):
                t = const.tile([128, C], BF16, tag=f"wt{ib}")
                nc.sync.dma_start(out=t, in_=wt_ext.ap()[ib * 128:(ib + 1) * 128, :])
                wt_sb.append(t)
            dr_sb = []
            for ot in range(2):
                t = const.tile([128, 6], F32, tag=f"dr{ot}")
                nc.sync.dma_start(out=t, in_=dr_ext.ap()[ot * 128:(ot + 1) * 128, :])
                dr_sb.append(t)

            import contextlib
            _loop = tc.For_i(0, reps, 1) if reps > 1 else contextlib.nullcontext()
            with _loop:
                _body(nc, tc, const, planes, psum, work, mmp,
                      e2pool if E2_BUFS else work,
                      sigpool if SIG_BUFS else work,
                      x_sb, wt_sb, dr_sb, x_ext, wt_ext, dr_ext, out_ext)

    nc.compile()
    return nc


def _body(nc, tc, const, planes, psum, work, mmp, e2pool, sigpool,
          x_sb, wt_sb, dr_sb, x_ext, wt_ext, dr_ext, out_ext):
            kvpe_ot, kvpo_ot = [], []
            for ot in range(2):
                # padded q plane, even phase: kvpe[:, r, j] = kvp[r, j]
                kvpe = planes.tile([128, H + 2, W + 2], BF16, tag="kvpe")
                # zero only the border (interior is fully overwritten)
                nc.gpsimd.memset(kvpe[:, 0, :], 0.0)
                nc.gpsimd.memset(kvpe[:, 65, :], 0.0)
                nc.gpsimd.memset(kvpe[:, 1:65, 0:1], 0.0)
                nc.gpsimd.memset(kvpe[:, 1:65, 65:66], 0.0)
                # odd phase: kvpo[:, r, j] = kvp[r, j+1]
                kvpo = planes.tile([128, H + 2, W + 2], BF16, tag="kvpo")

                # kvpo border: only row 0 / row 65 / col 64 matter (cols
                # [0,65) are written by the odd-phase copies below except the
                # border rows; col 64 = kvp col 65 = 0)
                nc.gpsimd.memset(kvpo[:, 0, :], 0.0)
                nc.gpsimd.memset(kvpo[:, 65, :], 0.0)
                nc.gpsimd.memset(kvpo[:, 1:65, 64:66], 0.0)

                # q = wt.T @ x in PSUM chunks of 512.  ib is the OUTER loop so
                # the stationary weights stay loaded for 8 consecutive
                # matmuls; 8 psum banks accumulate in parallel.
                pss = []
                for ck in range(HW // 512):
                    ps = psum.tile([128, 512], F32, tag="ps", name=f"ps{ot}_{ck}")
                    pss.append(ps)
                # ot0: ck-outer so each psum chunk finishes after 2
                # matmuls and the first copies/pointwise start early; ot1:
                # ib-outer (stationary reuse) since it overlaps ot0 pointwise
                mm_order = ([(ib, ck) for ck in range(HW // 512)
                             for ib in range(2)] if ot == 0 else
                            [(ib, ck) for ib in range(2)
                             for ck in range(HW // 512)])
                for (ib, ck) in mm_order:
                    nc.tensor.matmul(
                        pss[ck],
                        wt_sb[ib][:, ot * 128:(ot + 1) * 128],
                        x_sb[ib][:, ck * 512:(ck + 1) * 512],
                        start=(ib == 0),
                        stop=(ib == 1),
                    )
                for ck in range(HW // 512):
                    # even-phase copy into kvpe interior band
                    if KVPE_ENG == "pool":
                        nc.gpsimd.tensor_copy(
                            out=kvpe[:, 1 + ck * 8:1 + (ck + 1) * 8, 1:65],
                            in_=pss[ck].rearrange("p (r c) -> p r c", r=8),
                        )
                    else:
                        nc.scalar.activation(
                            out=kvpe[:, 1 + ck * 8:1 + (ck + 1) * 8, 1:65],
                            in_=pss[ck].rearrange("p (r c) -> p r c", r=8),
                            func=AF.Copy,
                        )
                    # odd phase band
                    if KVPO_MODE == "dma":
                        nc.sync.dma_start(
                            out=kvpo[:, 1 + ck * 8:1 + (ck + 1) * 8, 0:65],
                            in_=kvpe[:, 1 + ck * 8:1 + (ck + 1) * 8, 1:66],
                        )
                    elif KVPO_MODE == "act":
                        nc.scalar.activation(
                            out=kvpo[:, 1 + ck * 8:1 + (ck + 1) * 8, 0:64],
                            in_=pss[ck].rearrange("p (r c) -> p r c", r=8),
                            func=AF.Copy,
                        )
                    else:  # pool
                        nc.gpsimd.tensor_copy(
                            out=kvpo[:, 1 + ck * 8:1 + (ck + 1) * 8, 0:65],
                            in_=kvpe[:, 1 + ck * 8:1 + (ck + 1) * 8, 1:66],
                        )
                kvpe_ot.append(kvpe)
                kvpo_ot.append(kvpo)

                for hc in range(H // CH):
                    h0 = hc * CH
                    # difference planes for this chunk's row band
                    # [h0, h0+CH+2) (bf16 2x: all reads 4B-aligned)
                    # g1e[r,j] = kvp[r,j+1]-kvp[r,j]
                    # g1o[r,j] = kvp[r,j+2]-kvp[r,j+1]
                    # g2e[r,j] = kvp[r,j+2]-kvp[r,j]
                    kvpe_b = kvpe[:, h0:h0 + CH + 2, :]
                    kvpo_b = kvpo[:, h0:h0 + CH + 2, :]
                    g1e = work.tile([128, CH + 2, W + 2], BF16, tag="g1e")
                    g1o = work.tile([128, CH + 2, W + 2], BF16, tag="g1o")
                    g2e = work.tile([128, CH + 2, W + 2], BF16, tag="g2e")
                    nc.vector.tensor_sub(g1e[:, :, 0:64], kvpo_b[:, :, 0:64],
                                         kvpe_b[:, :, 0:64])
                    nc.vector.tensor_sub(g1o[:, :, 0:64], kvpe_b[:, :, 2:66],
                                         kvpo_b[:, :, 0:64])
                    nc.vector.tensor_sub(g2e[:, :, 0:64], kvpe_b[:, :, 2:66],
                                         kvpe_b[:, :, 0:64])
                    gmap = {"g1e": g1e, "g1o": g1o, "g2e": g2e}
                    # q[h,w] = kvp[h+1, w+1] = kvpo[h+1, w]
                    qv = kvpo[:, 1 + h0:1 + h0 + CH, 0:64]
                    sig_all = sigpool.tile([128, 3, CH, W], BF16, tag="sig")
                    for di in range(3):
                        # both pairs of this row batched through one tile:
                        # t2[:,k] = +-(G_k) + dr_k, then one broadcast mul by
                        # q, one exp, one fused sum+reciprocal.
                        row_pairs = [pr for pr in PAIRS if pr[1] == di]
                        t2 = work.tile([128, 2, CH, W], BF16, tag="t2")
                        for k, (p, pdi, pl, ro, co, pos) in enumerate(row_pairs):
                            gview = gmap[pl][:, ro:ro + CH, co:co + W]
                            if p in TS_ACT:
                                # +-G + dr as an ACT affine: Identity takes a
                                # per-partition bias AP and a scale immediate
                                nc.scalar.activation(
                                    out=t2[:, k], in_=gview, func=AF.Identity,
                                    scale=1.0 if pos else -1.0,
                                    bias=dr_sb[ot][:, p:p + 1],
                                )
                            elif pos:
                                nc.vector.tensor_scalar_add(
                                    t2[:, k], gview, dr_sb[ot][:, p:p + 1])
                            else:
                                # (G - dr) * -1 = dr - G, one 4x ts op
                                nc.vector.tensor_scalar(
                                    out=t2[:, k], in0=gview,
                                    scalar1=dr_sb[ot][:, p:p + 1],
                                    scalar2=-1.0,
                                    op0=OP.subtract, op1=OP.mult,
                                )
                        # one batched mul; q is free-dim-broadcast (stride 0)
                        # across the two pairs (HW-verified at 2x)
                        qb = bass.AP(tensor=qv.tensor, offset=qv.offset,
                                     ap=[qv.ap[0], [0, 2], qv.ap[1], qv.ap[2]])
                        a2 = work.tile([128, 2, CH, W], BF16, tag="a2")
                        nc.vector.tensor_mul(a2, t2, qb)
                        e2 = e2pool.tile([128, 2, CH, W], BF16, tag="e2")
                        nc.scalar.activation(out=e2, in_=a2, func=AF.Exp)
                        nc.vector._custom_dve(
                            SUMRECIP,
                            out=sig_all[:, di].rearrange("p r c -> p (r c)"),
                            in0=e2[:, 0].rearrange("p r c -> p (r c)"),
                            in1=e2[:, 1].rearrange("p r c -> p (r c)"),
                            s0=RECIP_C0, s1=RECIP_C1, imm2=1.0,
                        )
                    # final: out = sum_di sig_di * kvp[h+di, w+di].
                    # di 0 and 2 both read kvpe with a uniform 2*66+2 = 134
                    # element offset between them -> one batched 2x TT mul.
                    s02 = sig_all[:, 0:3:2]
                    v0 = kvpe[:, h0:h0 + CH, 0:64]
                    v02 = bass.AP(tensor=v0.tensor, offset=v0.offset,
                                  ap=[v0.ap[0], [134, 2], v0.ap[1], v0.ap[2]])
                    eng = {"pool": nc.gpsimd, "dve": nc.vector}
                    m02 = mmp.tile([128, 2, CH, W], BF16, tag="m02")
                    m02eng = nc.gpsimd if hc in M02_POOL_HC else nc.vector
                    m02eng.tensor_mul(m02, s02, v02)
                    m1 = mmp.tile([128, CH, W], BF16, tag="m1")
                    eng[M1_ENG].tensor_mul(m1, sig_all[:, 1],
                                           kvpo[:, 1 + h0:1 + h0 + CH, 0:64])
                    acc = mmp.tile([128, CH, W], BF16, tag="acc")
                    eng[ACC_ENG].tensor_add(acc, m02[:, 0], m02[:, 1])
                    accf = mmp.tile([128, CH, W], BF16, tag="accf")
                    eng[ACCF_ENG].tensor_add(accf, acc, m1)
                    nc.sync.dma_start(
                        out=out_ext.ap()[ot * 128:(ot + 1) * 128,
                                         h0 * W:(h0 + CH) * W],
                        in_=accf.rearrange("p r c -> p (r c)"),
                    )





_CACHE = {}


def _get_nc():
    if "nc" not in _CACHE:
        _CACHE["nc"] = _build()
    return _CACHE["nc"]


def _prep_in_maps(x, W_, rel):
    import ml_dtypes
    bf16 = ml_dtypes.bfloat16
    wt = np.ascontiguousarray(W_.T.astype(bf16))  # [cin, cout]
    r = rel.reshape(C, 3, 3).astype(np.float32)
    pairs = [(0, 1), (0, 2), (1, 0), (1, 2), (2, 0), (2, 1)]
    dr = np.stack([r[:, di, dj] - r[:, di, di] for (di, dj) in pairs], axis=1)
    dr = np.ascontiguousarray(dr.astype(np.float32))  # [C, 6]
    in_maps = []
    for c in range(NCORES):
        in_maps.append({
            "x": np.ascontiguousarray(x[c].reshape(C, HW).astype(bf16)),
            "wt": wt,
            "dr": dr,
        })
    return in_maps


def kernel(x, W, rel):
    nc = _get_nc()
    in_maps = _prep_in_maps(x, W, rel)
    res = run_bass_kernel_spmd(nc, in_maps, core_ids=list(range(NCORES)))
    out = np.stack([
        res.results[c]["out"].astype(np.float32).reshape(C, H, 64)
        for c in range(NCORES)
    ])
    return out.astype(np.float32)

